# revision 10
# baseline (speedup 1.0000x reference)
"""Causal multi-head attention (B=4, S=2048, D=1024, H=16, hd=64) on 8 TRN2
NeuronCores.

Sharding: core c = (batch b = c//2, head-group g = c%2). Each core computes
QKV projections for its 8 heads (Megatron column-split), causal attention,
and a partial out-projection (row-split); the host sums the two head-group
partials per batch and adds the bias.

On-device layout (bf16 compute, fp32 PSUM accumulation):
  xT  [p, q-block, din-subtile, 512]  x[b]^T pre-tiled on host so each
        input DMA moves 8KB-contiguous runs per partition (descriptor-
        efficient); same for wq/wk (head-pair-blocked), wv, wo, masks
  qT/kT as [d_g, S] transposed tiles: head-pair t -> partitions
        [0:64] head 2t, [64:128] head 2t+1
  v   [k-tile 128, 8 heads, 65]: col 64 is ones (sumexp lands in the ctx^T
        psum row 64 for free during the attn*V matmul)
  scores^T psum tiles [k 128, 2 heads, q 512] (2 banks): head pair packed
        via PE row tiling (K=64 each, concurrent), one exp over both
  attn = exp(scores/8), no max-subtraction (|s|/8 <= ~3), causal handled by
        skipping k-tiles above the diagonal, restricting the q-range on
        diagonal tiles (s0 = dd*128), and a mask multiply for the boundary
  ctx^T accumulated in PSUM over k-tiles; normalize via DRAM-roundtrip
        reciprocal + gpsimd partition broadcast.

Schedule: the attention stream is ACT(exp)-bound per-k-tile while the
projections/out-projection are pure PE work, so all non-attention matmuls
drain as filler INSIDE the attention stream, paced by interpolating between
per-chunk prerequisite markers with a 2-k-tile lead. Row 3's out-projection
is split: hp 0..2 partial sums land in SBUF during the last chunk; only one
matmul + add + DMA per o-tile remains after the final normalize. Scalar
engine runs exp only (plus t=0 DMA triggers).
"""

import numpy as np
import ml_dtypes

import concourse.bass as bass
import concourse.tile as tile
from concourse import bacc, mybir
from concourse.bass_utils import run_bass_kernel_spmd

P = 128          # partitions
S = 2048         # sequence length (one batch per core)
DIN = 1024       # model dim
DG = 512         # head-group width per core (8 heads x 64)
HD = 64          # head dim
NH = 8           # heads per core
QC = 512         # q-chunk (matmul free dim)
NQC = S // QC    # 4 q-chunks
NKT = S // P     # 16 k-tiles
KDT = DIN // P   # 8 din k-tiles
NHP = 4          # head pairs per core
F32 = mybir.dt.float32
F32R = mybir.dt.float32r
BF16 = mybir.dt.bfloat16
EXP = mybir.ActivationFunctionType.Exp

_CACHE = {}


def _emit(tc, d):
    nc = tc.nc
    with (
        nc.allow_low_precision(reason="bf16 attention pipeline"),
        tc.tile_pool(name="persist", bufs=1) as pp,
        tc.tile_pool(name="work", bufs=4) as wp,
        tc.tile_pool(name="psc", bufs=2, space="PSUM") as psc,
        tc.tile_pool(name="ppj", bufs=2, space="PSUM") as ppj,
        tc.tile_pool(name="pcx", bufs=1, space="PSUM") as pcx,
    ):
        # ---- persistent SBUF tiles (layouts match the pre-tiled DRAM) ----
        xT = pp.tile([P, NQC, KDT, QC], BF16, tag="xT", name="xT")
        wq = pp.tile([P, NHP, KDT, P], BF16, tag="wq", name="wq")
        wk = pp.tile([P, NHP, KDT, P], BF16, tag="wk", name="wk")
        wv = pp.tile([P, KDT, DG], BF16, tag="wv", name="wv")
        wo = pp.tile([P, 4, DIN], BF16, tag="wo", name="wo")
        qT = [pp.tile([P, S], BF16, tag=f"qT{t}", name=f"qT{t}") for t in range(NHP)]
        kT = [pp.tile([P, S], BF16, tag=f"kT{t}", name=f"kT{t}") for t in range(NHP)]
        vv = [pp.tile([P, NH, HD + 1], BF16, tag=f"v{m}", name=f"v{m}") for m in range(NKT)]
        cx = [pp.tile([P, S], BF16, tag=f"cx{t}", name=f"cx{t}") for t in range(NHP)]
        ob3 = [pp.tile([P, QC], F32, tag=f"ob3{o}", name=f"ob3{o}") for o in range(8)]
        msk = pp.tile([P, 4, 2, QC], BF16, tag="msk", name="msk")

        # ---- input DMAs: big contiguous-run transfers, ordered by need,
        # alternating the two HW-DGE rings (sync / scalar-at-t=0-only) ----
        nc.sync.dma_start(xT[:, 0, :, :], d["xT"][:, 0, :, :])
        nc.scalar.dma_start(wq[:, 0, :, :], d["wqT"][:, 0, :, :])
        nc.scalar.dma_start(wk[:, 0, :, :], d["wkT"][:, 0, :, :])
        nc.sync.dma_start(wv[:], d["wvT"][:])
        nc.scalar.dma_start(msk[:], d["masks"][:])
        nc.scalar.dma_start(wq[:, 1:NHP, :, :], d["wqT"][:, 1:NHP, :, :])
        nc.sync.dma_start(xT[:, 1, :, :], d["xT"][:, 1, :, :])
        nc.scalar.dma_start(wk[:, 1:NHP, :, :], d["wkT"][:, 1:NHP, :, :])
        nc.sync.dma_start(xT[:, 2, :, :], d["xT"][:, 2, :, :])
        nc.scalar.dma_start(wo[:], d["woT"][:])
        nc.sync.dma_start(xT[:, 3, :, :], d["xT"][:, 3, :, :])

        # ---- filler units (each ~4-8 matmuls of PE work) ----
        def u_v(m):
            def f():
                ps = ppj.tile([P, QC], F32, tag="pj", name="ps")
                for k in range(KDT):
                    nc.tensor.matmul(
                        ps[:],
                        xT[:, m // 4, k, (m % 4) * P:(m % 4 + 1) * P],
                        wv[:, k, :],
                        start=(k == 0),
                        stop=(k == KDT - 1),
                    )
                nc.vector.tensor_copy(
                    vv[m][:, :, 0:HD], ps[:].rearrange("p (h e) -> p h e", h=NH)
                )
                nc.vector.memset(vv[m][:, :, HD:HD + 1], 1.0)
            return f

        def u_chain(t, w, s):
            def f():
                wt, dst = ((wq, qT), (wk, kT))[w]
                ps = ppj.tile([P, QC], F32, tag="pj", name="ps")
                for k in range(KDT):
                    nc.tensor.matmul(
                        ps[:],
                        wt[:, t, k, :],
                        xT[:, s, k, :],
                        start=(k == 0),
                        stop=(k == KDT - 1),
                    )
                nc.vector.tensor_copy(dst[t][:, s * QC:(s + 1) * QC], ps[:])
            return f

        def u_out(s, o):
            def f():
                ps = ppj.tile([P, QC], F32, tag="pj", name="ps")
                for k in range(4):
                    nc.tensor.matmul(
                        ps[:],
                        wo[:, k, o * P:(o + 1) * P],
                        cx[k][:, s * QC:(s + 1) * QC],
                        start=(k == 0), stop=(k == 3),
                    )
                ob = wp.tile([P, QC], F32, tag="ob", name="ob")
                nc.vector.tensor_copy(ob[:], ps[:])
                nc.sync.dma_start(
                    d["outT"][o * P:(o + 1) * P, s * QC:(s + 1) * QC], ob[:]
                )
            return f

        def u_out3_partial(o):
            # row-3 out-proj, head-pair groups 0..2 only -> SBUF partial
            def f():
                ps = ppj.tile([P, QC], F32, tag="pj", name="ps")
                for k in range(3):
                    nc.tensor.matmul(
                        ps[:],
                        wo[:, k, o * P:(o + 1) * P],
                        cx[k][:, 3 * QC:S],
                        start=(k == 0), stop=(k == 2),
                    )
                nc.vector.tensor_copy(ob3[o][:], ps[:])
            return f

        def u_out3_final(o):
            ps = ppj.tile([P, QC], F32, tag="pj", name="ps")
            nc.tensor.matmul(
                ps[:], wo[:, 3, o * P:(o + 1) * P], cx[3][:, 3 * QC:S],
                start=True, stop=True,
            )
            ob = wp.tile([P, QC], F32, tag="ob", name="ob")
            nc.vector.tensor_add(ob[:], ps[:], ob3[o][:])
            nc.sync.dma_start(d["outT"][o * P:(o + 1) * P, 3 * QC:S], ob[:])

        # consume-ordered filler queue + hard prerequisites per chunk
        # (prefill covers chunk (0,0)'s chains + V m0,m1; m2,m3 drain first)
        queue = [u_v(2), u_v(3)]
        pre = {}
        for s in range(NQC):
            for hp in range(NHP):
                if (hp, s) == (0, 0):
                    pre[(hp, s)] = 0
                    continue
                if hp == 0 and s >= 1:
                    queue += [u_v(m) for m in range(4 * s, 4 * s + 4)]
                queue += [u_chain(hp, 0, s), u_chain(hp, 1, s)]
                pre[(hp, s)] = len(queue)
            if s >= 1:
                queue += [u_out(s - 1, o) for o in range(8)]
        queue += [u_out3_partial(o) for o in range(8)]
        n_units = len(queue)

        # next-chunk marker for interpolated filler draining: during chunk c,
        # drain linearly from pre[c] to pre[next(c)], 2 k-tiles ahead so the
        # next chunk's chains land before its first scores matmul
        order = [(hp, s) for s in range(NQC) for hp in range(NHP)]
        nxt = {order[i]: order[i + 1] for i in range(len(order) - 1)}

        state = {"drained": 0}

        def drain_to(idx):
            while state["drained"] < idx:
                queue[state["drained"]]()
                state["drained"] += 1

        def attn_chunk(hp, s):
            t0 = pre[(hp, s)]
            t1 = pre[nxt[(hp, s)]] if (hp, s) in nxt else n_units
            nkt = 4 * (s + 1)  # causal: k-tiles 0..nkt-1
            cps = pcx.tile([HD + 1, 2, QC], F32, tag="cx", name="cps")
            for k in range(nkt):
                dd = k - 4 * s
                s0 = max(dd, 0) * P  # causal q-range restriction
                sps = psc.tile([P, 2, QC], F32, tag="sc", name="sps")
                nc.tensor.matmul(
                    sps[:, 0, s0:],
                    kT[hp][0:HD, k * P:(k + 1) * P],
                    qT[hp][0:HD, s * QC + s0:(s + 1) * QC],
                    start=True, stop=True,
                )
                nc.tensor.matmul(
                    sps[:, 1, s0:],
                    kT[hp][HD:P, k * P:(k + 1) * P],
                    qT[hp][HD:P, s * QC + s0:(s + 1) * QC],
                    start=True, stop=True,
                )
                a = wp.tile([P, 2, QC], BF16, tag="a", name="a", bufs=6)
                nc.scalar.activation(
                    a[:, :, s0:], sps[:, :, s0:], EXP, scale=0.125
                )
                if dd >= 0:
                    # only columns [s0, s0+128) straddle the diagonal
                    nc.vector.tensor_mul(
                        a[:, :, s0:s0 + P], a[:, :, s0:s0 + P],
                        msk[:, dd, :, s0:s0 + P],
                    )
                nc.tensor.matmul(
                    cps[:, 0, s0:], vv[k][:, 2 * hp, :], a[:, 0, s0:],
                    start=(k == 0), stop=(k == nkt - 1),
                )
                nc.tensor.matmul(
                    cps[:, 1, s0:], vv[k][:, 2 * hp + 1, :], a[:, 1, s0:],
                    start=(k == 0), stop=(k == nkt - 1),
                )
                drain_to(min(t1, t0 + ((t1 - t0) * (k + 3)) // nkt))
            # normalize: rows 0:64 are ctx^T, row 64 is sumexp
            cb = wp.tile([HD + 1, 2, QC], F32, tag="cb", name="cb", bufs=2)
            nc.vector.tensor_copy(cb[:], cps[:])
            # reciprocal of the [1, 1024] sumexp row with free-dim 8:
            # reshape to [128, 8] via SBUF-SBUF DMA so the DVE iterative
            # divide (8 cyc/elem along free dim) runs on free-dim 8
            zt = wp.tile([P, 8], F32, tag="zt", name="zt", bufs=2)
            nc.sync.dma_start(zt[:], cb[HD:HD + 1, :, :])
            rt = wp.tile([P, 8], F32, tag="rt", name="rt", bufs=2)
            nc.vector.reciprocal(rt[:], zt[:])
            rc = wp.tile([P, 2, QC], F32, tag="rc", name="rc", bufs=2)
            nc.sync.dma_start(rc[0:1, :, :], rt[:])
            bs = wp.tile([HD, 2, QC], F32, tag="bs", name="bs", bufs=2)
            nc.gpsimd.partition_broadcast(bs[:], rc[0:1, :, :])
            # head B first: its partition-shift DMA overlaps head A's mul
            cxs = wp.tile([HD, QC], BF16, tag="cxs", name="cxs", bufs=2)
            nc.vector.tensor_mul(cxs[:], cb[0:HD, 1, :], bs[:, 1, :])
            nc.sync.dma_start(
                cx[hp][HD:P, s * QC:(s + 1) * QC], cxs[:]
            )
            nc.vector.tensor_mul(
                cx[hp][0:HD, s * QC:(s + 1) * QC],
                cb[0:HD, 0, :], bs[:, 0, :],
            )

        # ---- prefill: just enough to start chunk (0,0) ----
        u_chain(0, 0, 0)()
        u_chain(0, 1, 0)()
        u_v(0)()
        u_v(1)()

        # ---- main stream ----
        for s in range(NQC):
            for hp in range(NHP):
                drain_to(pre[(hp, s)])
                attn_chunk(hp, s)
        drain_to(n_units)
        for o in range(8):
            u_out3_final(o)


def _build():
    if "nc" in _CACHE:
        return _CACHE["nc"]
    nc = bacc.Bacc("TRN2", target_bir_lowering=False, debug=False, num_devices=8)
    d = {
        "xT": nc.dram_tensor("xT", [P, NQC, KDT, QC], BF16, kind="ExternalInput").ap(),
        "wqT": nc.dram_tensor("wqT", [P, NHP, KDT, P], BF16, kind="ExternalInput").ap(),
        "wkT": nc.dram_tensor("wkT", [P, NHP, KDT, P], BF16, kind="ExternalInput").ap(),
        "wvT": nc.dram_tensor("wvT", [P, KDT, DG], BF16, kind="ExternalInput").ap(),
        "woT": nc.dram_tensor("woT", [P, 4, DIN], BF16, kind="ExternalInput").ap(),
        "masks": nc.dram_tensor("masks", [P, 4, 2, QC], BF16, kind="ExternalInput").ap(),
        "outT": nc.dram_tensor("outT", [DIN, S], F32, kind="ExternalOutput").ap(),
    }
    with tile.TileContext(nc) as tc:
        _emit(tc, d)
    nc.compile()
    _CACHE["nc"] = nc
    return nc


def _masks_np():
    r = np.arange(P)[:, None]
    j = np.arange(QC)[None, :]
    m = np.concatenate(
        [(j >= r + dd * P).astype(ml_dtypes.bfloat16) for dd in range(4)], axis=1
    )  # [128, 4*512]
    m = m.reshape(P, 4, 1, QC)
    return np.ascontiguousarray(np.broadcast_to(m, (P, 4, 2, QC)))


def _tile_k(a, kdt=KDT):
    """[kdt*P, C] -> [P, kdt, C] (din-subtile blocking)."""
    c = a.shape[1]
    return np.ascontiguousarray(a.reshape(kdt, P, c).transpose(1, 0, 2))


def kernel(x, Wq, Wk, Wv, Wo, bo, _run_kwargs=None, _return_res=False):
    x = np.asarray(x)
    Wq, Wk, Wv, Wo, bo = (np.asarray(a) for a in (Wq, Wk, Wv, Wo, bo))
    B = x.shape[0]
    nc = _build()

    def b16(a):
        return np.ascontiguousarray(a).astype(ml_dtypes.bfloat16)

    masks = _masks_np()
    in_maps = []
    for c in range(8):
        b, g = divmod(c, 2)
        xt = b16(x[b].T)  # [1024, 2048]
        # [p, q-block, k-subtile, 512]
        xt = xt.reshape(KDT, P, NQC, QC).transpose(1, 2, 0, 3)
        wqt = b16(Wq[g * DG:(g + 1) * DG, :].T)  # [1024, 512]
        wkt = b16(Wk[g * DG:(g + 1) * DG, :].T)
        # [p, head-pair, k-subtile, 128]
        wqt = wqt.reshape(KDT, P, NHP, P).transpose(1, 2, 0, 3)
        wkt = wkt.reshape(KDT, P, NHP, P).transpose(1, 2, 0, 3)
        in_maps.append({
            "xT": np.ascontiguousarray(xt),
            "wqT": np.ascontiguousarray(wqt),
            "wkT": np.ascontiguousarray(wkt),
            "wvT": _tile_k(b16(Wv[g * DG:(g + 1) * DG, :].T)),
            "woT": _tile_k(b16(Wo[:, g * DG:(g + 1) * DG].T), kdt=4),
            "masks": masks,
        })

    res = run_bass_kernel_spmd(nc, in_maps, list(range(8)), **(_run_kwargs or {}))
    out = np.empty((B, S, DIN), np.float32)
    for b in range(B):
        p = res.results[2 * b]["outT"] + res.results[2 * b + 1]["outT"]
        out[b] = p.T + bo.astype(np.float32)
    if _return_res:
        return out, res
    return out


# revision 17
# speedup vs baseline: 1.0015x; 1.0015x over previous
"""Causal multi-head attention (B=4, S=2048, D=1024, H=16, hd=64) on 8 TRN2
NeuronCores.

Sharding: core c = (batch b = c//2, head-group g = c%2). Each core computes
QKV projections for its 8 heads (Megatron column-split), causal attention,
and a partial out-projection (row-split); the host sums the two head-group
partials per batch and adds the bias.

On-device layout (bf16 compute, fp32 PSUM accumulation):
  xT  [p, q-block, din-subtile, 512]  x[b]^T pre-tiled on host so each
        input DMA moves 8KB-contiguous runs per partition (descriptor-
        efficient); same for wq/wk (head-pair-blocked), wv, wo, masks
  qT/kT as [d_g, S] transposed tiles: head-pair t -> partitions
        [0:64] head 2t, [64:128] head 2t+1
  v   [k-tile 128, 8 heads, 65]: col 64 is ones (sumexp lands in the ctx^T
        psum row 64 for free during the attn*V matmul)
  scores^T psum tiles [k 128, 2 heads, q 512] (2 banks): head pair packed
        via PE row tiling (K=64 each, concurrent), one exp over both
  attn = exp(scores/8), no max-subtraction (|s|/8 <= ~3), causal handled by
        skipping k-tiles above the diagonal, restricting the q-range on
        diagonal tiles (s0 = dd*128), and a mask multiply for the boundary
  ctx^T accumulated in PSUM over k-tiles; normalize via DRAM-roundtrip
        reciprocal + gpsimd partition broadcast.

Schedule: the attention stream is ACT(exp)-bound per-k-tile while the
projections/out-projection are pure PE work, so all non-attention matmuls
drain as filler INSIDE the attention stream, paced by interpolating between
per-chunk prerequisite markers with a 2-k-tile lead. Row 3's out-projection
is split: hp 0..2 partial sums land in SBUF during the last chunk; only one
matmul + add + DMA per o-tile remains after the final normalize. Scalar
engine runs exp only (plus t=0 DMA triggers).
"""

import numpy as np
import ml_dtypes

import concourse.bass as bass
import concourse.tile as tile
from concourse import bacc, mybir
from concourse.bass_utils import run_bass_kernel_spmd

P = 128          # partitions
S = 2048         # sequence length (one batch per core)
DIN = 1024       # model dim
DG = 512         # head-group width per core (8 heads x 64)
HD = 64          # head dim
NH = 8           # heads per core
QC = 512         # q-chunk (matmul free dim)
NQC = S // QC    # 4 q-chunks
NKT = S // P     # 16 k-tiles
KDT = DIN // P   # 8 din k-tiles
NHP = 4          # head pairs per core
F32 = mybir.dt.float32
F32R = mybir.dt.float32r
BF16 = mybir.dt.bfloat16
EXP = mybir.ActivationFunctionType.Exp

_CACHE = {}


def _emit(tc, d):
    nc = tc.nc
    with (
        nc.allow_low_precision(reason="bf16 attention pipeline"),
        tc.tile_pool(name="persist", bufs=1) as pp,
        tc.tile_pool(name="work", bufs=4) as wp,
        tc.tile_pool(name="psc", bufs=2, space="PSUM") as psc,
        tc.tile_pool(name="ppj", bufs=2, space="PSUM") as ppj,
        tc.tile_pool(name="pcx", bufs=1, space="PSUM") as pcx,
    ):
        # ---- persistent SBUF tiles (layouts match the pre-tiled DRAM) ----
        xT = pp.tile([P, NQC, KDT, QC], BF16, tag="xT", name="xT")
        wq = pp.tile([P, NHP, KDT, P], BF16, tag="wq", name="wq")
        wk = pp.tile([P, NHP, KDT, P], BF16, tag="wk", name="wk")
        wv = pp.tile([P, KDT, DG], BF16, tag="wv", name="wv")
        wo = pp.tile([P, 4, DIN], BF16, tag="wo", name="wo")
        qT = [pp.tile([P, S], BF16, tag=f"qT{t}", name=f"qT{t}") for t in range(NHP)]
        kT = [pp.tile([P, S], BF16, tag=f"kT{t}", name=f"kT{t}") for t in range(NHP)]
        vv = [pp.tile([P, NH, HD + 1], BF16, tag=f"v{m}", name=f"v{m}") for m in range(NKT)]
        cx = [pp.tile([P, S], BF16, tag=f"cx{t}", name=f"cx{t}") for t in range(NHP)]
        ob3 = [pp.tile([P, QC], F32, tag=f"ob3{o}", name=f"ob3{o}") for o in range(8)]
        msk = pp.tile([P, 4, QC], BF16, tag="msk", name="msk")

        # ---- input DMAs: big contiguous-run transfers, ordered by need,
        # critical prefill set first across both HW-DGE rings ----
        nc.sync.dma_start(xT[:, 0, :, :], d["xT"][:, 0, :, :])
        nc.scalar.dma_start(wq[:, 0, :, :], d["wqT"][:, 0, :, :])
        nc.scalar.dma_start(wk[:, 0, :, :], d["wkT"][:, 0, :, :])
        nc.scalar.dma_start(msk[:], d["masks"][:])
        nc.sync.dma_start(wv[:], d["wvT"][:])
        nc.scalar.dma_start(wq[:, 1:NHP, :, :], d["wqT"][:, 1:NHP, :, :])
        nc.sync.dma_start(xT[:, 1, :, :], d["xT"][:, 1, :, :])
        nc.scalar.dma_start(wk[:, 1:NHP, :, :], d["wkT"][:, 1:NHP, :, :])
        nc.sync.dma_start(xT[:, 2, :, :], d["xT"][:, 2, :, :])
        nc.scalar.dma_start(wo[:], d["woT"][:])
        nc.sync.dma_start(xT[:, 3, :, :], d["xT"][:, 3, :, :])

        # ---- filler units (each ~4-8 matmuls of PE work) ----
        def u_v(m):
            def f():
                ps = ppj.tile([P, QC], F32, tag="pj", name="ps")
                for k in range(KDT):
                    nc.tensor.matmul(
                        ps[:],
                        xT[:, m // 4, k, (m % 4) * P:(m % 4 + 1) * P],
                        wv[:, k, :],
                        start=(k == 0),
                        stop=(k == KDT - 1),
                    )
                nc.vector.tensor_copy(
                    vv[m][:, :, 0:HD], ps[:].rearrange("p (h e) -> p h e", h=NH)
                )
                nc.vector.memset(vv[m][:, :, HD:HD + 1], 1.0)
            return f

        def u_chain(t, w, s):
            def f():
                wt, dst = ((wq, qT), (wk, kT))[w]
                ps = ppj.tile([P, QC], F32, tag="pj", name="ps")
                for k in range(KDT):
                    nc.tensor.matmul(
                        ps[:],
                        wt[:, t, k, :],
                        xT[:, s, k, :],
                        start=(k == 0),
                        stop=(k == KDT - 1),
                    )
                nc.vector.tensor_copy(dst[t][:, s * QC:(s + 1) * QC], ps[:])
            return f

        def u_out(s, o):
            def f():
                ps = ppj.tile([P, QC], F32, tag="pj", name="ps")
                for k in range(4):
                    nc.tensor.matmul(
                        ps[:],
                        wo[:, k, o * P:(o + 1) * P],
                        cx[k][:, s * QC:(s + 1) * QC],
                        start=(k == 0), stop=(k == 3),
                    )
                ob = wp.tile([P, QC], F32, tag="ob", name="ob")
                nc.vector.tensor_copy(ob[:], ps[:])
                nc.sync.dma_start(
                    d["outT"][o * P:(o + 1) * P, s * QC:(s + 1) * QC], ob[:]
                )
            return f

        def u_out3_partial(o):
            # row-3 out-proj, head-pair groups 0..2 only -> SBUF partial
            def f():
                ps = ppj.tile([P, QC], F32, tag="pj", name="ps")
                for k in range(3):
                    nc.tensor.matmul(
                        ps[:],
                        wo[:, k, o * P:(o + 1) * P],
                        cx[k][:, 3 * QC:S],
                        start=(k == 0), stop=(k == 2),
                    )
                nc.vector.tensor_copy(ob3[o][:], ps[:])
            return f

        def u_out3_final(o):
            ps = ppj.tile([P, QC], F32, tag="pj", name="ps")
            nc.tensor.matmul(
                ps[:], wo[:, 3, o * P:(o + 1) * P], cx[3][:, 3 * QC:S],
                start=True, stop=True,
            )
            ob = wp.tile([P, QC], F32, tag="ob", name="ob")
            nc.vector.tensor_add(ob[:], ps[:], ob3[o][:])
            nc.sync.dma_start(d["outT"][o * P:(o + 1) * P, 3 * QC:S], ob[:])

        # consume-ordered filler queue + hard prerequisites per chunk
        # (prefill covers only chunk (0,0)'s chains; V m0..3 drain first)
        queue = [u_v(0), u_v(1), u_v(2), u_v(3)]
        pre = {}
        for s in range(NQC):
            for hp in range(NHP):
                if (hp, s) == (0, 0):
                    pre[(hp, s)] = 0
                    continue
                if hp == 0 and s >= 1:
                    queue += [u_v(m) for m in range(4 * s, 4 * s + 4)]
                queue += [u_chain(hp, 0, s), u_chain(hp, 1, s)]
                pre[(hp, s)] = len(queue)
            if s >= 1:
                queue += [u_out(s - 1, o) for o in range(8)]
        n_units = len(queue)

        # next-chunk marker for interpolated filler draining: during chunk c,
        # drain linearly from pre[c] to pre[next(c)], 2 k-tiles ahead so the
        # next chunk's chains land before its first scores matmul
        order = [(hp, s) for s in range(NQC) for hp in range(NHP)]
        nxt = {order[i]: order[i + 1] for i in range(len(order) - 1)}

        state = {"drained": 0}

        def drain_to(idx):
            while state["drained"] < idx:
                queue[state["drained"]]()
                state["drained"] += 1

        def attn_chunk(hp, s):
            t0 = pre[(hp, s)]
            t1 = pre[nxt[(hp, s)]] if (hp, s) in nxt else n_units
            nkt = 4 * (s + 1)  # causal: k-tiles 0..nkt-1
            cps = pcx.tile([HD + 1, 2, QC], F32, tag="cx", name="cps")

            def attn_v(k, s0, a):
                nc.tensor.matmul(
                    cps[:, 0, s0:], vv[k][:, 2 * hp, :], a[:, 0, s0:],
                    start=(k == 0), stop=(k == nkt - 1),
                )
                nc.tensor.matmul(
                    cps[:, 1, s0:], vv[k][:, 2 * hp + 1, :], a[:, 1, s0:],
                    start=(k == 0), stop=(k == nkt - 1),
                )

            pend = None  # emit attn*V one k-tile late: by the time it reaches
            # the head of the (FIFO) tensor queue its exp has finished, so it
            # never head-blocks the scores stream behind it
            for k in range(nkt):
                dd = k - 4 * s
                s0 = max(dd, 0) * P  # causal q-range restriction
                sps = psc.tile([P, 2, QC], F32, tag="sc", name="sps")
                nc.tensor.matmul(
                    sps[:, 0, s0:],
                    kT[hp][0:HD, k * P:(k + 1) * P],
                    qT[hp][0:HD, s * QC + s0:(s + 1) * QC],
                    start=True, stop=True,
                )
                nc.tensor.matmul(
                    sps[:, 1, s0:],
                    kT[hp][HD:P, k * P:(k + 1) * P],
                    qT[hp][HD:P, s * QC + s0:(s + 1) * QC],
                    start=True, stop=True,
                )
                a = wp.tile([P, 2, QC], BF16, tag="a", name="a", bufs=6)
                nc.scalar.activation(
                    a[:, :, s0:], sps[:, :, s0:], EXP, scale=0.125
                )
                if dd >= 0:
                    # only columns [s0, s0+128) straddle the diagonal
                    for h in range(2):
                        nc.vector.tensor_mul(
                            a[:, h, s0:s0 + P], a[:, h, s0:s0 + P],
                            msk[:, dd, s0:s0 + P],
                        )
                if pend is not None:
                    attn_v(*pend)
                pend = (k, s0, a)
                drain_to(min(t1, t0 + ((t1 - t0) * (k + 3)) // nkt))
            attn_v(*pend)
            # normalize: rows 0:64 are ctx^T, row 64 is sumexp
            cb = wp.tile([HD + 1, 2, QC], F32, tag="cb", name="cb", bufs=2)
            nc.vector.tensor_copy(cb[:], cps[:])
            # reciprocal of the [1, 1024] sumexp row with free-dim 8:
            # reshape to [128, 8] via SBUF-SBUF DMA so the DVE iterative
            # divide (8 cyc/elem along free dim) runs on free-dim 8
            zt = wp.tile([P, 8], F32, tag="zt", name="zt", bufs=2)
            nc.sync.dma_start(zt[:], cb[HD:HD + 1, :, :])
            rt = wp.tile([P, 8], F32, tag="rt", name="rt", bufs=2)
            nc.vector.reciprocal(rt[:], zt[:])
            rc = wp.tile([P, 2, QC], F32, tag="rc", name="rc", bufs=2)
            nc.sync.dma_start(rc[0:1, :, :], rt[:])
            bs = wp.tile([HD, 2, QC], F32, tag="bs", name="bs", bufs=2)
            nc.gpsimd.partition_broadcast(bs[:], rc[0:1, :, :])
            # head B first: its partition-shift DMA overlaps head A's mul
            cxs = wp.tile([HD, QC], BF16, tag="cxs", name="cxs", bufs=2)
            nc.vector.tensor_mul(cxs[:], cb[0:HD, 1, :], bs[:, 1, :])
            nc.sync.dma_start(
                cx[hp][HD:P, s * QC:(s + 1) * QC], cxs[:]
            )
            nc.vector.tensor_mul(
                cx[hp][0:HD, s * QC:(s + 1) * QC],
                cb[0:HD, 0, :], bs[:, 0, :],
            )

        # ---- prefill: just enough to start chunk (0,0) ----
        u_chain(0, 0, 0)()
        u_chain(0, 1, 0)()

        # ---- main stream ----
        for s in range(NQC):
            for hp in range(NHP):
                drain_to(pre[(hp, s)])
                attn_chunk(hp, s)
        drain_to(n_units)
        # row-3 out-proj partials run during chunk (3,3)'s normalize chain,
        # keeping the PE busy (and HAM warm) until the finals
        for o in range(8):
            u_out3_partial(o)()
        for o in range(8):
            u_out3_final(o)


def _build():
    if "nc" in _CACHE:
        return _CACHE["nc"]
    nc = bacc.Bacc("TRN2", target_bir_lowering=False, debug=False, num_devices=8)
    d = {
        "xT": nc.dram_tensor("xT", [P, NQC, KDT, QC], BF16, kind="ExternalInput").ap(),
        "wqT": nc.dram_tensor("wqT", [P, NHP, KDT, P], BF16, kind="ExternalInput").ap(),
        "wkT": nc.dram_tensor("wkT", [P, NHP, KDT, P], BF16, kind="ExternalInput").ap(),
        "wvT": nc.dram_tensor("wvT", [P, KDT, DG], BF16, kind="ExternalInput").ap(),
        "woT": nc.dram_tensor("woT", [P, 4, DIN], BF16, kind="ExternalInput").ap(),
        "masks": nc.dram_tensor("masks", [P, 4, QC], BF16, kind="ExternalInput").ap(),
        "outT": nc.dram_tensor("outT", [DIN, S], F32, kind="ExternalOutput").ap(),
    }
    with tile.TileContext(nc) as tc:
        _emit(tc, d)
    nc.compile()
    _CACHE["nc"] = nc
    return nc


def _masks_np():
    r = np.arange(P)[:, None]
    j = np.arange(QC)[None, :]
    m = np.stack(
        [(j >= r + dd * P).astype(ml_dtypes.bfloat16) for dd in range(4)], axis=1
    )  # [128, 4, 512]
    return np.ascontiguousarray(m)


def _tile_k(a, kdt=KDT):
    """[kdt*P, C] -> [P, kdt, C] (din-subtile blocking)."""
    c = a.shape[1]
    return np.ascontiguousarray(a.reshape(kdt, P, c).transpose(1, 0, 2))


def kernel(x, Wq, Wk, Wv, Wo, bo, _run_kwargs=None, _return_res=False):
    x = np.asarray(x)
    Wq, Wk, Wv, Wo, bo = (np.asarray(a) for a in (Wq, Wk, Wv, Wo, bo))
    B = x.shape[0]
    nc = _build()

    def b16(a):
        return np.ascontiguousarray(a).astype(ml_dtypes.bfloat16)

    masks = _masks_np()
    in_maps = []
    for c in range(8):
        b, g = divmod(c, 2)
        xt = b16(x[b].T)  # [1024, 2048]
        # [p, q-block, k-subtile, 512]
        xt = xt.reshape(KDT, P, NQC, QC).transpose(1, 2, 0, 3)
        wqt = b16(Wq[g * DG:(g + 1) * DG, :].T)  # [1024, 512]
        wkt = b16(Wk[g * DG:(g + 1) * DG, :].T)
        # [p, head-pair, k-subtile, 128]
        wqt = wqt.reshape(KDT, P, NHP, P).transpose(1, 2, 0, 3)
        wkt = wkt.reshape(KDT, P, NHP, P).transpose(1, 2, 0, 3)
        in_maps.append({
            "xT": np.ascontiguousarray(xt),
            "wqT": np.ascontiguousarray(wqt),
            "wkT": np.ascontiguousarray(wkt),
            "wvT": _tile_k(b16(Wv[g * DG:(g + 1) * DG, :].T)),
            "woT": _tile_k(b16(Wo[:, g * DG:(g + 1) * DG].T), kdt=4),
            "masks": masks,
        })

    res = run_bass_kernel_spmd(nc, in_maps, list(range(8)), **(_run_kwargs or {}))
    out = np.empty((B, S, DIN), np.float32)
    for b in range(B):
        p = res.results[2 * b]["outT"] + res.results[2 * b + 1]["outT"]
        out[b] = p.T + bo.astype(np.float32)
    if _return_res:
        return out, res
    return out


# revision 24
# speedup vs baseline: 1.0523x; 1.0508x over previous
"""Causal multi-head attention (B=4, S=2048, D=1024, H=16, hd=64) on 8 TRN2
NeuronCores.

Sharding: core c = (batch b = c//2, head-group g = c%2). Each core computes
QKV projections for its 8 heads (Megatron column-split), causal attention,
and a partial out-projection (row-split); the host sums the two head-group
partials per batch and adds the bias.

On-device layout (bf16 compute, fp32 PSUM accumulation):
  xT  [p, q-block, din-subtile, 512]  x[b]^T pre-tiled on host so each
        input DMA moves 8KB-contiguous runs per partition (descriptor-
        efficient); same for wq/wk (head-pair-blocked), wv, wo, masks
  q/k projections in fp8-e4m3 DoubleRow (weights pre-scaled x64 on host,
        exp scale absorbs the 1/4096; rel err ~1.7e-2 vs 2e-2 budget)
  qT/kT as [d_g, S] transposed tiles: head-pair t -> partitions
        [0:64] head 2t, [64:128] head 2t+1
  v   [k-tile 128, 8 heads, 65]: col 64 is ones (sumexp lands in the ctx^T
        psum row 64 for free during the attn*V matmul)
  scores^T psum tiles [k 128, 2 heads, q 512] (2 banks): head pair packed
        via PE row tiling (K=64 each, concurrent), one exp over both
  attn = exp(scores/8), causal via skipping k-tiles above the diagonal,
        restricting the q-range on diagonal tiles, and a mask multiply
  ctx^T accumulated in PSUM over k-tiles; normalize via DRAM-roundtrip
        reciprocal + gpsimd partition broadcast (DVE 32x32-transpose
        reciprocal for the last chunk to cut the tail latency).

Schedule: all non-attention matmuls drain as filler INSIDE the attention
stream (interpolated between per-chunk prerequisite markers, 2 k-tiles of
lead); attn*V is emitted one k-tile late so it never head-blocks the FIFO
tensor queue; dummy warm-up matmuls keep/get HAM to full clock during the
initial DMA wait; row-3 out-projection is split so only one matmul + add +
DMA per o-tile remains after the final normalize, with spare out-proj row-2
units retained to keep the PE warm through that normalize.
"""

import numpy as np
import ml_dtypes

import concourse.bass as bass
import concourse.tile as tile
from concourse import bacc, mybir
from concourse.bass_utils import run_bass_kernel_spmd

P = 128          # partitions
S = 2048         # sequence length (one batch per core)
DIN = 1024       # model dim
DG = 512         # head-group width per core (8 heads x 64)
HD = 64          # head dim
NH = 8           # heads per core
QC = 512         # q-chunk (matmul free dim)
NQC = S // QC    # 4 q-chunks
NKT = S // P     # 16 k-tiles
KDT = DIN // P   # 8 din k-tiles
NHP = 4          # head pairs per core
F32 = mybir.dt.float32
BF16 = mybir.dt.bfloat16
FP8 = mybir.dt.float8e4
EXP = mybir.ActivationFunctionType.Exp
DR = mybir.MatmulPerfMode.DoubleRow

USE_FP8_QK = True   # fp8 DoubleRow q/k projections (x64 weight pre-scale)
WSCALE = 64.0
N_WARM = 14         # dummy warm-up matmuls during the initial DMA wait

_CACHE = {}


def _emit(tc, d):
    nc = tc.nc
    with (
        nc.allow_low_precision(reason="bf16 attention pipeline"),
        tc.tile_pool(name="persist", bufs=1) as pp,
        tc.tile_pool(name="work", bufs=4) as wp,
        tc.tile_pool(name="psc", bufs=2, space="PSUM") as psc,
        tc.tile_pool(name="ppj", bufs=2, space="PSUM") as ppj,
        tc.tile_pool(name="pcx", bufs=1, space="PSUM") as pcx,
    ):
        # ---- persistent SBUF tiles (layouts match the pre-tiled DRAM) ----
        xT = pp.tile([P, NQC, KDT, QC], BF16, tag="xT", name="xT")
        if USE_FP8_QK:
            x8 = pp.tile([P, NQC, KDT, QC], FP8, tag="x8", name="x8")
            wq = pp.tile([P, NHP, KDT, P], FP8, tag="wq", name="wq")
            wk = pp.tile([P, NHP, KDT, P], FP8, tag="wk", name="wk")
        else:
            wq = pp.tile([P, NHP, KDT, P], BF16, tag="wq", name="wq")
            wk = pp.tile([P, NHP, KDT, P], BF16, tag="wk", name="wk")
        wv = pp.tile([P, KDT, DG], BF16, tag="wv", name="wv")
        wo = pp.tile([P, 4, DIN], BF16, tag="wo", name="wo")
        qT = [pp.tile([P, S], BF16, tag=f"qT{t}", name=f"qT{t}") for t in range(NHP)]
        kT = [pp.tile([P, S], BF16, tag=f"kT{t}", name=f"kT{t}") for t in range(NHP)]
        vv = [pp.tile([P, NH, HD + 1], BF16, tag=f"v{m}", name=f"v{m}") for m in range(NKT)]
        cx = [pp.tile([P, S], BF16, tag=f"cx{t}", name=f"cx{t}") for t in range(NHP)]
        OB3T = BF16 if USE_FP8_QK else F32
        ob3 = [pp.tile([P, QC], OB3T, tag=f"ob3{o}", name=f"ob3{o}") for o in range(8)]
        msk = pp.tile([P, 4, QC], BF16, tag="msk", name="msk")
        wrm = pp.tile([P, QC], BF16, tag="wrm", name="wrm")

        # ---- PE warm-up: garbage matmuls keep the PE busy (HAM at full
        # clock) while the first input DMAs land ----
        nc.vector.memset(wrm[:], 0.0)
        for g in range(0, N_WARM, 7):
            ps = ppj.tile([P, QC], F32, tag="pj", name="ps")
            n = min(7, N_WARM - g)
            for i in range(n):
                nc.tensor.matmul(
                    ps[:], wrm[:, 0:P], wrm[:],
                    start=(i == 0), stop=(i == n - 1),
                )

        # ---- input DMAs: big contiguous-run transfers, ordered by need,
        # critical prefill set first across both HW-DGE rings ----
        xq = x8 if USE_FP8_QK else xT
        nc.sync.dma_start(xq[:, 0, 0:4, :], d["xq"][:, 0, 0:4, :])
        nc.scalar.dma_start(wq[:, 0, :, :], d["wqT"][:, 0, :, :])
        nc.scalar.dma_start(wk[:, 0, :, :], d["wkT"][:, 0, :, :])
        nc.sync.dma_start(xq[:, 0, 4:KDT, :], d["xq"][:, 0, 4:KDT, :])
        nc.scalar.dma_start(msk[:], d["masks"][:])
        if USE_FP8_QK:
            nc.sync.dma_start(xT[:, 0, :, :], d["xT"][:, 0, :, :])
        nc.sync.dma_start(wv[:], d["wvT"][:])
        nc.scalar.dma_start(wq[:, 1:NHP, :, :], d["wqT"][:, 1:NHP, :, :])
        nc.scalar.dma_start(wk[:, 1:NHP, :, :], d["wkT"][:, 1:NHP, :, :])
        for s in range(1, NQC):
            nc.sync.dma_start(xT[:, s, :, :], d["xT"][:, s, :, :])
            if USE_FP8_QK:
                nc.scalar.dma_start(x8[:, s, :, :], d["xq"][:, s, :, :])
        nc.scalar.dma_start(wo[:], d["woT"][:])

        # ---- filler units ----
        def u_v(m):
            def f():
                ps = ppj.tile([P, QC], F32, tag="pj", name="ps")
                for k in range(KDT):
                    nc.tensor.matmul(
                        ps[:],
                        xT[:, m // 4, k, (m % 4) * P:(m % 4 + 1) * P],
                        wv[:, k, :],
                        start=(k == 0),
                        stop=(k == KDT - 1),
                    )
                nc.vector.tensor_copy(
                    vv[m][:, :, 0:HD], ps[:].rearrange("p (h e) -> p h e", h=NH)
                )
                nc.vector.memset(vv[m][:, :, HD:HD + 1], 1.0)
            return f

        def u_chain(t, w, s):
            def f():
                wt, dst = ((wq, qT), (wk, kT))[w]
                ps = ppj.tile([P, QC], F32, tag="pj", name="ps")
                if USE_FP8_QK:
                    for k in range(0, KDT, 2):
                        nc.tensor.matmul(
                            ps[:],
                            wt[:, t, k:k + 2, :],
                            x8[:, s, k:k + 2, :],
                            start=(k == 0),
                            stop=(k == KDT - 2),
                            perf_mode=DR,
                        )
                else:
                    for k in range(KDT):
                        nc.tensor.matmul(
                            ps[:],
                            wt[:, t, k, :],
                            xT[:, s, k, :],
                            start=(k == 0),
                            stop=(k == KDT - 1),
                        )
                nc.vector.tensor_copy(dst[t][:, s * QC:(s + 1) * QC], ps[:])
            return f

        def u_out(s, o):
            def f():
                ps = ppj.tile([P, QC], F32, tag="pj", name="ps")
                for k in range(4):
                    nc.tensor.matmul(
                        ps[:],
                        wo[:, k, o * P:(o + 1) * P],
                        cx[k][:, s * QC:(s + 1) * QC],
                        start=(k == 0), stop=(k == 3),
                    )
                ob = wp.tile([P, QC], F32, tag="ob", name="ob", bufs=2)
                nc.vector.tensor_copy(ob[:], ps[:])
                nc.sync.dma_start(
                    d["outT"][o * P:(o + 1) * P, s * QC:(s + 1) * QC], ob[:]
                )
            return f

        def u_out3_partial(o):
            # row-3 out-proj, head-pair groups 0..2 only -> SBUF partial
            def f():
                ps = ppj.tile([P, QC], F32, tag="pj", name="ps")
                for k in range(3):
                    nc.tensor.matmul(
                        ps[:],
                        wo[:, k, o * P:(o + 1) * P],
                        cx[k][:, 3 * QC:S],
                        start=(k == 0), stop=(k == 2),
                    )
                nc.vector.tensor_copy(ob3[o][:], ps[:])
            return f

        def u_out3_final(o):
            ps = ppj.tile([P, QC], F32, tag="pj", name="ps")
            nc.tensor.matmul(
                ps[:], wo[:, 3, o * P:(o + 1) * P], cx[3][:, 3 * QC:S],
                start=True, stop=True,
            )
            ob = wp.tile([P, QC], F32, tag="ob", name="ob", bufs=2)
            nc.vector.tensor_add(ob[:], ps[:], ob3[o][:])
            nc.sync.dma_start(d["outT"][o * P:(o + 1) * P, 3 * QC:S], ob[:])

        # consume-ordered filler queue + hard prerequisites per chunk
        queue = [u_v(0), u_v(1), u_v(2), u_v(3)]
        pre = {}
        for s in range(NQC):
            for hp in range(NHP):
                if (hp, s) == (0, 0):
                    pre[(hp, s)] = 0
                    continue
                if hp == 0 and s >= 1:
                    queue += [u_v(m) for m in range(4 * s, 4 * s + 4)]
                queue += [u_chain(hp, 0, s), u_chain(hp, 1, s)]
                pre[(hp, s)] = len(queue)
            if s == 1 or s == 2:
                queue += [u_out(s - 1, o) for o in range(8)]
        # keep 4 of row-2's out-proj units back: they run during chunk
        # (3,3)'s final normalize chain, keeping the PE warm
        queue += [u_out(2, o) for o in range(4)]
        n_units = len(queue)

        order = [(hp, s) for s in range(NQC) for hp in range(NHP)]
        nxt = {order[i]: order[i + 1] for i in range(len(order) - 1)}

        state = {"drained": 0}

        def drain_to(idx):
            while state["drained"] < idx:
                queue[state["drained"]]()
                state["drained"] += 1

        def normalize(hp, s):
            last = (hp, s) == (NHP - 1, NQC - 1)
            cb = wp.tile([96, 2, QC], F32, tag="cb", name="cb", bufs=2)
            cps = state["cps"]
            nc.vector.tensor_copy(cb[0:HD + 1], cps[:])
            if last:
                # low-latency path: DVE 32x32 block-transpose reshapes the
                # [1,1024] sumexp row (row 0 of the 32-aligned cb[64:96]
                # window; rows 65:96 are filler) onto 32 partitions,
                # iterative reciprocal on free-dim 32, transpose back -
                # no SBUF-DMA roundtrips in the tail
                nc.vector.tensor_copy(cb[64:96], cps[0:32])
                nc.vector.tensor_copy(cb[64:65], cps[HD:HD + 1])
                t1 = wp.tile([32, 2, QC], F32, tag="t1", name="t1", bufs=1)
                nc.vector.transpose(t1[:], cb[64:96])
                t2 = wp.tile([32, 2, QC], F32, tag="t2", name="t2", bufs=1)
                nc.vector.tensor_copy(t2[:], t1[:])
                tv1 = t1[:].rearrange("p h (b j) -> p (h b) j", j=32)
                tv2 = t2[:].rearrange("p h (b j) -> p (h b) j", j=32)
                nc.vector.reciprocal(tv2[:, :, 0:1], tv1[:, :, 0:1])
                rc = wp.tile([32, 2, QC], F32, tag="rc", name="rc", bufs=2)
                nc.vector.transpose(rc[:], t2[:])
                rrow = rc[0:1, :, :]
            else:
                zt = wp.tile([P, 8], F32, tag="zt", name="zt", bufs=2)
                nc.sync.dma_start(zt[:], cb[HD:HD + 1, :, :])
                rt = wp.tile([P, 8], F32, tag="rt", name="rt", bufs=2)
                nc.vector.reciprocal(rt[:], zt[:])
                rc = wp.tile([32, 2, QC], F32, tag="rc", name="rc", bufs=2)
                nc.sync.dma_start(rc[0:1, :, :], rt[:])
                rrow = rc[0:1, :, :]
            bs = wp.tile([HD, 2, QC], F32, tag="bs", name="bs", bufs=2)
            nc.gpsimd.partition_broadcast(bs[:], rrow)
            # head B first: its partition-shift DMA overlaps head A's mul
            cxs = wp.tile([HD, QC], BF16, tag="cxs", name="cxs", bufs=2)
            nc.vector.tensor_mul(cxs[:], cb[0:HD, 1, :], bs[:, 1, :])
            nc.sync.dma_start(cx[hp][HD:P, s * QC:(s + 1) * QC], cxs[:])
            nc.vector.tensor_mul(
                cx[hp][0:HD, s * QC:(s + 1) * QC], cb[0:HD, 0, :], bs[:, 0, :]
            )

        def attn_chunk(hp, s):
            t0 = pre[(hp, s)]
            t1 = pre[nxt[(hp, s)]] if (hp, s) in nxt else n_units
            nkt = 4 * (s + 1)  # causal: k-tiles 0..nkt-1
            cps = pcx.tile([HD + 1, 2, QC], F32, tag="cx", name="cps")
            state["cps"] = cps

            def attn_v(k, s0, a):
                nc.tensor.matmul(
                    cps[:, 0, s0:], vv[k][:, 2 * hp, :], a[:, 0, s0:],
                    start=(k == 0), stop=(k == nkt - 1),
                )
                nc.tensor.matmul(
                    cps[:, 1, s0:], vv[k][:, 2 * hp + 1, :], a[:, 1, s0:],
                    start=(k == 0), stop=(k == nkt - 1),
                )

            pend = None  # attn*V emitted one k-tile late: by the time it
            # reaches the head of the FIFO tensor queue its exp is done, so
            # it never head-blocks the scores stream behind it
            for k in range(nkt):
                dd = k - 4 * s
                s0 = max(dd, 0) * P  # causal q-range restriction
                sps = psc.tile([P, 2, QC], F32, tag="sc", name="sps")
                nc.tensor.matmul(
                    sps[:, 0, s0:],
                    kT[hp][0:HD, k * P:(k + 1) * P],
                    qT[hp][0:HD, s * QC + s0:(s + 1) * QC],
                    start=True, stop=True,
                )
                nc.tensor.matmul(
                    sps[:, 1, s0:],
                    kT[hp][HD:P, k * P:(k + 1) * P],
                    qT[hp][HD:P, s * QC + s0:(s + 1) * QC],
                    start=True, stop=True,
                )
                a = wp.tile([P, 2, QC], BF16, tag="a", name="a", bufs=5)
                nc.scalar.activation(
                    a[:, :, s0:], sps[:, :, s0:], EXP, scale=d["escale"]
                )
                if dd >= 0:
                    # only columns [s0, s0+128) straddle the diagonal
                    for h in range(2):
                        nc.vector.tensor_mul(
                            a[:, h, s0:s0 + P], a[:, h, s0:s0 + P],
                            msk[:, dd, s0:s0 + P],
                        )
                if pend is not None:
                    attn_v(*pend)
                pend = (k, s0, a)
                drain_to(min(t1, t0 + ((t1 - t0) * (k + 3)) // nkt))
            attn_v(*pend)
            normalize(hp, s)

        # ---- prefill: just enough to start chunk (0,0) ----
        u_chain(0, 0, 0)()
        u_chain(0, 1, 0)()

        # ---- main stream ----
        for s in range(NQC):
            for hp in range(NHP):
                drain_to(pre[(hp, s)])
                attn_chunk(hp, s)
        # row-3 out-proj partials + held-back row-2 units run during chunk
        # (3,3)'s normalize chain, keeping the PE busy (and HAM warm)
        drain_to(n_units)
        for o in range(8):
            u_out3_partial(o)()
        for o in range(4, 8):
            u_out(2, o)()
        for o in range(8):
            u_out3_final(o)


def _build():
    if "nc" in _CACHE:
        return _CACHE["nc"]
    nc = bacc.Bacc("TRN2", target_bir_lowering=False, debug=False, num_devices=8)
    d = {
        "xT": nc.dram_tensor("xT", [P, NQC, KDT, QC], BF16, kind="ExternalInput").ap(),
        "wvT": nc.dram_tensor("wvT", [P, KDT, DG], BF16, kind="ExternalInput").ap(),
        "woT": nc.dram_tensor("woT", [P, 4, DIN], BF16, kind="ExternalInput").ap(),
        "masks": nc.dram_tensor("masks", [P, 4, QC], BF16, kind="ExternalInput").ap(),
        "outT": nc.dram_tensor("outT", [DIN, S], F32, kind="ExternalOutput").ap(),
    }
    wdt = FP8 if USE_FP8_QK else BF16
    d["wqT"] = nc.dram_tensor("wqT", [P, NHP, KDT, P], wdt, kind="ExternalInput").ap()
    d["wkT"] = nc.dram_tensor("wkT", [P, NHP, KDT, P], wdt, kind="ExternalInput").ap()
    if USE_FP8_QK:
        d["xq"] = nc.dram_tensor("xq", [P, NQC, KDT, QC], FP8, kind="ExternalInput").ap()
        d["escale"] = 0.125 / (WSCALE * WSCALE)
    else:
        d["xq"] = d["xT"]
        d["escale"] = 0.125
    with tile.TileContext(nc) as tc:
        _emit(tc, d)
    nc.compile()
    _CACHE["nc"] = nc
    return nc


def _masks_np():
    r = np.arange(P)[:, None]
    j = np.arange(QC)[None, :]
    m = np.stack(
        [(j >= r + dd * P).astype(ml_dtypes.bfloat16) for dd in range(4)], axis=1
    )  # [128, 4, 512]
    return np.ascontiguousarray(m)


def _tile_k(a, kdt=KDT):
    """[kdt*P, C] -> [P, kdt, C] (din-subtile blocking)."""
    c = a.shape[1]
    return np.ascontiguousarray(a.reshape(kdt, P, c).transpose(1, 0, 2))


def _f8(a):
    return np.clip(a, -240, 240).astype(ml_dtypes.float8_e4m3)


def kernel(x, Wq, Wk, Wv, Wo, bo, _run_kwargs=None, _return_res=False):
    x = np.asarray(x)
    Wq, Wk, Wv, Wo, bo = (np.asarray(a) for a in (Wq, Wk, Wv, Wo, bo))
    B = x.shape[0]
    nc = _build()

    def b16(a):
        return np.ascontiguousarray(a).astype(ml_dtypes.bfloat16)

    masks = _masks_np()
    in_maps = []
    for c in range(8):
        b, g = divmod(c, 2)
        xt = b16(x[b].T)  # [1024, 2048]
        xt4 = xt.reshape(KDT, P, NQC, QC).transpose(1, 2, 0, 3)  # [p,s,k,c]
        wqt = Wq[g * DG:(g + 1) * DG, :].T  # [1024, 512] f32
        wkt = Wk[g * DG:(g + 1) * DG, :].T
        im = {
            "xT": np.ascontiguousarray(xt4),
            "wvT": _tile_k(b16(Wv[g * DG:(g + 1) * DG, :].T)),
            "woT": _tile_k(b16(Wo[:, g * DG:(g + 1) * DG].T), kdt=4),
            "masks": masks,
        }
        if USE_FP8_QK:
            im["xq"] = np.ascontiguousarray(
                _f8(x[b].T).reshape(KDT, P, NQC, QC).transpose(1, 2, 0, 3))
            im["wqT"] = np.ascontiguousarray(
                _f8(WSCALE * wqt).reshape(KDT, P, NHP, P).transpose(1, 2, 0, 3))
            im["wkT"] = np.ascontiguousarray(
                _f8(WSCALE * wkt).reshape(KDT, P, NHP, P).transpose(1, 2, 0, 3))
        else:
            im["wqT"] = np.ascontiguousarray(
                b16(wqt).reshape(KDT, P, NHP, P).transpose(1, 2, 0, 3))
            im["wkT"] = np.ascontiguousarray(
                b16(wkt).reshape(KDT, P, NHP, P).transpose(1, 2, 0, 3))
        in_maps.append(im)

    res = run_bass_kernel_spmd(nc, in_maps, list(range(8)), **(_run_kwargs or {}))
    out = np.empty((B, S, DIN), np.float32)
    for b in range(B):
        p = res.results[2 * b]["outT"] + res.results[2 * b + 1]["outT"]
        out[b] = p.T + bo.astype(np.float32)
    if _return_res:
        return out, res
    return out
